# revision 7
# baseline (speedup 1.0000x reference)
"""Multi-head attention (B=4, H=16, S=2048, D=128, causal+pad mask) on 8 TRN2 NeuronCores.

Sharding: the 64 (batch, head) pairs are split 8 per core (pure data parallel —
attention is independent per head, no collectives needed).

Per-core kernel (per head):
  - scores are computed TRANSPOSED: S^T[k, q] = K_block^T^T @ Q^T with the
    contraction dim d=128 on partitions, the k-block (128) as the PSUM partition
    dim and the allowed q-columns (128-chunk granular, derived from the actual
    mask) as the moving dim. Q/K are host-cast to bf16.
  - The allowed 128x128 chunks are packed into PSUM group tiles of up to 12
    chunks ([128, 1536] f32 = 3 banks) so ONE scalar-engine ACTIVATE computes
    exp(scale*s) for the whole group out of PSUM into SBUF bf16 — the ACT
    engine is the critical path (1 col/cycle @1.2GHz), so group count is
    minimized. Bins never span more than 2 q-megatiles so only 2 PV output
    tiles are ever live. No max-subtraction: scores*scale ~ N(0,1), exp is safe.
  - Partially-masked 128x128 chunks are zeroed by a bf16 multiply with
    host-derived deduped mask tiles on the vector engine. Fully-masked chunks
    are never computed; fully-allowed chunks are untouched.
  - P^T lands exactly in the layout the PV matmul needs (k on partitions):
    O[q_sub 128, 132] += P^T[:, chunk]^T @ V'[k_block] accumulated over k
    blocks in PSUM, where V' is V in bf16 with a ones column appended at col
    128 — so O[:, 128] is the softmax denominator for free. Both q_subs of a
    256-wide megatile share one PSUM bank ([128, 2, 132]).
  - reciprocal + per-partition scale normalizes, then DMA out as f32.
  - ALL heads' inputs are DMA-prefetched into SBUF up front (~120KB of the
    208KB/partition budget) so no mid-kernel DMA wait ever stalls compute;
    head 0 is quartered so the first matmul can start early.
"""

import os
import sys
from collections import defaultdict

import numpy as np

try:  # the repo root that provides `concourse` / `gauge`
    import concourse.bass  # noqa: F401
except ImportError:  # pragma: no cover
    for _p in ("/opt/trn_rl_repo", "/root/.axon_site/_ro/trn_rl_repo"):
        if os.path.isdir(_p) and _p not in sys.path:
            sys.path.insert(0, _p)

import ml_dtypes

B, H, S, D = 4, 16, 2048, 128
BH = B * H
NCORES = 8
HPC = BH // NCORES  # heads per core = 8
QM = 256  # q megatile width; q sub-chunks of 128 map to PV output tiles
CH = 128  # q chunk granularity (PV stationary width / mask tile width)
KB = 128  # k block (PSUM partition dim of S^T)
NM = S // QM  # 8 q megatiles
NKB = S // KB  # 16 k blocks
VW = D + 4  # V' width: col D holds ones (softmax denom), cols D+1.. are zero pad
GCH = 12  # exp group size in chunks: [128, 1536] f32 = 3 PSUM banks
SCALE = float(np.float32(1.0 / np.sqrt(np.float32(D))))
NSUB = QM // CH  # q sub-chunks per megatile = 2

_CACHE: dict = {}
LAST_RESULTS = None  # BassKernelResults of the most recent run (for test harness)


def _derive_schedule(attn_mask):
    """Derive the chunk-level block schedule from the actual mask.

    Returns (bins, contrib, mask_tiles):
      bins: list of exp-group bins; each bin is an ordered list of chunks
        (m, j, lo, mask_id_or_None) where lo is the 128-aligned q-column
        offset within megatile m and mask_id indexes mask_tiles (None = fully
        allowed). Bins hold <= GCH chunks and never span more than 2 distinct
        megatiles (so only 2 PV accumulators are live at once).
      contrib: {(m, sub): n} count of PV contributions per output sub-tile.
      mask_tiles: [128, n_masks, CH] bf16 deduped transposed chunk masks.
    """
    am = np.asarray(attn_mask) != 0  # [S(q), S(k)]
    uniq: dict = {}
    tiles = []
    chunks = []  # (m, j, lo, mask_id|None)
    contrib: dict = defaultdict(int)
    for m in range(NM):
        for j in range(NKB):
            for c in range(NSUB):
                cm = am[m * QM + c * CH : m * QM + (c + 1) * CH, j * KB : (j + 1) * KB]
                if not cm.any():
                    continue
                if cm.all():
                    mid = None
                else:
                    key = cm.tobytes()
                    if key not in uniq:
                        uniq[key] = len(tiles)
                        tiles.append(cm.T.astype(ml_dtypes.bfloat16))  # [KB, CH]
                    mid = uniq[key]
                chunks.append((m, j, c * CH, mid))
                contrib[(m, c)] += 1

    # pack chunks into bins of exactly GCH (one ACT instruction each)
    bins = [chunks[i : i + GCH] for i in range(0, len(chunks), GCH)]

    mask_tiles = np.stack(tiles, axis=1) if tiles else None  # [128, n, CH]
    return bins, dict(contrib), mask_tiles


def _build_program(bins, contrib, n_masks, use_pad):
    import concourse.mybir as mybir
    import concourse.tile as tile
    from concourse import bacc

    f32 = mybir.dt.float32
    bf16 = mybir.dt.bfloat16
    Exp = mybir.ActivationFunctionType.Exp

    GCOLS = GCH * CH

    nc = bacc.Bacc(None)
    qt_ext = nc.declare_dram_parameter("qt", [HPC, 128, S], bf16, isOutput=False)
    kt_ext = nc.declare_dram_parameter("kt", [HPC, 128, S], bf16, isOutput=False)
    vp_ext = nc.declare_dram_parameter("vp", [HPC, 128, NKB, VW], bf16, isOutput=False)
    if n_masks:
        mk_ext = nc.declare_dram_parameter("mk", [128, n_masks, CH], bf16, isOutput=False)
    if use_pad:
        pc_ext = nc.declare_dram_parameter("pc", [128, NKB], f32, isOutput=False)
    out_ext = nc.declare_dram_parameter("out", [HPC, S, D], f32, isOutput=True)

    with tile.TileContext(nc) as tc:
        with (
            tc.tile_pool(name="qt", bufs=HPC) as qt_pool,
            tc.tile_pool(name="kt", bufs=HPC) as kt_pool,
            tc.tile_pool(name="vp", bufs=HPC) as vp_pool,
            tc.tile_pool(name="pt", bufs=6) as pt_pool,
            tc.tile_pool(name="osb", bufs=6) as osb_pool,
            tc.tile_pool(name="rec", bufs=4) as rec_pool,
            tc.tile_pool(name="mk", bufs=1) as mk_pool,
            tc.tile_pool(name="warm", bufs=1) as warm_pool,
            tc.tile_pool(name="st", bufs=2, space="PSUM") as st_pool,
            tc.tile_pool(name="ops", bufs=2, space="PSUM") as o_pool,
        ):
            # mask / pad-column loads first — tiny, and PV needs them early
            if n_masks:
                mk = mk_pool.tile([128, n_masks, CH], bf16)
                nc.sync.dma_start(mk[:], mk_ext[:])
            if use_pad:
                pc = mk_pool.tile([128, NKB], f32)
                nc.sync.dma_start(pc[:], pc_ext[:])

            # prefetch ALL heads' inputs into SBUF up front. head 0 is
            # quartered so its first k/q columns land fast; later heads use
            # single whole-tensor transfers (4KB+/partition — DMA-efficient).
            qts, kts, vps = [], [], []
            for h in range(HPC):
                qts.append(qt_pool.tile([128, S], bf16, name="qt"))
                kts.append(kt_pool.tile([128, S], bf16, name="kt"))
                vps.append(vp_pool.tile([128, NKB, VW], bf16, name="vp"))
            NQ = 4
            qs = S // NQ
            js = NKB // NQ
            for q4 in range(NQ):
                nc.sync.dma_start(
                    kts[0][:, q4 * qs : (q4 + 1) * qs],
                    kt_ext[0, :, q4 * qs : (q4 + 1) * qs],
                )
                nc.sync.dma_start(
                    qts[0][:, q4 * qs : (q4 + 1) * qs],
                    qt_ext[0, :, q4 * qs : (q4 + 1) * qs],
                )
                nc.sync.dma_start(
                    vps[0][:, q4 * js : (q4 + 1) * js, :],
                    vp_ext[0, :, q4 * js : (q4 + 1) * js, :],
                )
            for h in range(1, HPC):
                nc.sync.dma_start(kts[h][:], kt_ext[h])
                nc.sync.dma_start(qts[h][:], qt_ext[h])
                nc.sync.dma_start(vps[h][:], vp_ext[h])

            # PE warm-up: ~3us of dummy matmuls during the DMA prologue trips
            # the HAM clock gate to 2.4 GHz before the first real matmul
            warm = warm_pool.tile([128, 512], bf16, name="warm")
            nc.gpsimd.memset(warm[:], 0.0)
            wo = st_pool.tile([128, GCOLS], f32, tag="st", name="wo")
            for wi in range(7):
                nc.tensor.matmul(
                    wo[:, 0:512], lhsT=warm[:, 0:128], rhs=warm[:], start=True, stop=True
                )

            for h in range(HPC):
                qt, kt, vp = qts[h], kts[h], vps[h]
                o_tiles: dict = {}
                seen: dict = defaultdict(int)
                deferred: dict = defaultdict(list)

                def finalize(m, sub, o):
                    rec = rec_pool.tile([128, 1], f32, name="rec")
                    nc.vector.reciprocal(rec[:], o[:, D : D + 1])
                    osb = osb_pool.tile([128, D], f32, name="osb")
                    nc.vector.tensor_scalar_mul(osb[:], o[:, 0:D], rec[:])
                    row0 = m * QM + sub * CH
                    nc.sync.dma_start(out_ext[h, row0 : row0 + CH, :], osb[:])

                def emit_pv(m, sub, j, pt, pcol):
                    # PSUM banks support only ONE open accumulation group at a
                    # time, so each (m, sub) group runs in its own 1-bank tile;
                    # sub1's group (same pool slot cycle) opens only after
                    # sub0's closes (see emit_bin's deferral).
                    key = (m, sub)
                    if key not in o_tiles:
                        o_tiles[key] = o_pool.tile([128, VW], f32, tag="o", name="o")
                    seen[key] += 1
                    nc.tensor.matmul(
                        o_tiles[key][:],
                        lhsT=pt[:, pcol : pcol + CH],
                        rhs=vp[:, j, :],
                        start=seen[key] == 1,
                        stop=seen[key] == contrib[key],
                    )
                    if seen[key] == contrib[key]:
                        finalize(m, sub, o_tiles.pop(key))
                        return True
                    return False

                def emit_bin(bn):
                    gcols = len(bn) * CH
                    st = st_pool.tile([128, GCOLS], f32, tag="st", name="st")
                    # scores: coalesce consecutive chunks of the same (m, j)
                    # into one matmul, splitting at PSUM 512-col banks
                    p = 0
                    while p < len(bn):
                        m, j, lo, _ = bn[p]
                        p2 = p + 1
                        while (
                            p2 < len(bn)
                            and bn[p2][0] == m
                            and bn[p2][1] == j
                            and bn[p2][2] == bn[p2 - 1][2] + CH
                        ):
                            p2 += 1
                        w = (p2 - p) * CH
                        off = 0
                        while off < w:
                            pcol = p * CH + off
                            wseg = min(w - off, 512 - pcol % 512)
                            nc.tensor.matmul(
                                st[:, pcol : pcol + wseg],
                                lhsT=kt[:, j * KB : (j + 1) * KB],
                                rhs=qt[:, m * QM + lo + off : m * QM + lo + off + wseg],
                                start=True,
                                stop=True,
                            )
                            off += wseg
                        p = p2
                    pt = pt_pool.tile([128, GCOLS], bf16, tag="pt", name="pt")
                    nc.scalar.activation(pt[:, :gcols], st[:, :gcols], Exp, scale=SCALE)

                    # mask/pad fixups in place, then PV. sub 0 accumulates as
                    # chunks arrive; sub 1 chunks are deferred until sub 0's
                    # accumulation group closes, so only one group per megatile
                    # (plus the neighbor's) is ever open -> 2 PSUM banks.
                    for p, (m, j, lo, mid) in enumerate(bn):
                        pcol = p * CH
                        if mid is not None:
                            nc.vector.tensor_mul(
                                pt[:, pcol : pcol + CH],
                                pt[:, pcol : pcol + CH],
                                mk[:, mid, :],
                            )
                        if use_pad:
                            nc.vector.tensor_scalar_mul(
                                pt[:, pcol : pcol + CH],
                                pt[:, pcol : pcol + CH],
                                pc[:, j : j + 1],
                            )
                        sub = lo // CH
                        if sub == 0:
                            if emit_pv(m, 0, j, pt, pcol):
                                for dm, dj, dpt, dpcol in deferred.pop(m, []):
                                    emit_pv(dm, 1, dj, dpt, dpcol)
                        elif (m, 0) in o_tiles or seen[(m, 0)] < contrib.get(
                            (m, 0), 0
                        ):
                            deferred[m].append((m, j, pt, pcol))
                        else:
                            emit_pv(m, 1, j, pt, pcol)

                for bn in bins:
                    emit_bin(bn)
    nc.compile()
    return nc


def _prep_inputs(q, k, v, attn_mask, pad_mask):
    q = np.asarray(q, dtype=np.float32).reshape(BH, S, D)
    k = np.asarray(k, dtype=np.float32).reshape(BH, S, D)
    v = np.asarray(v, dtype=np.float32).reshape(BH, S, D)

    qt = np.ascontiguousarray(q.transpose(0, 2, 1)).astype(ml_dtypes.bfloat16)
    kt = np.ascontiguousarray(k.transpose(0, 2, 1)).astype(ml_dtypes.bfloat16)

    # V': [BH, 128(row within k block), NKB, VW] bf16; col D = 1.0 (denominator)
    vp = np.zeros((BH, 128, NKB, VW), dtype=ml_dtypes.bfloat16)
    vblocks = v.reshape(BH, NKB, 128, D).transpose(0, 2, 1, 3)
    vp[:, :, :, :D] = vblocks.astype(ml_dtypes.bfloat16)
    vp[:, :, :, D] = 1.0

    pad = np.asarray(pad_mask).reshape(B, S) != 0
    use_pad = not bool(pad.all())
    pcs = None
    if use_pad:
        pcs = []
        for c in range(NCORES):
            b = (c * HPC) // H
            pcs.append(
                np.ascontiguousarray(pad[b].reshape(NKB, 128).T.astype(np.float32))
            )
    return qt, kt, vp, use_pad, pcs


def kernel(q, k, v, attn_mask, pad_mask):
    global LAST_RESULTS
    from concourse.bass_utils import run_bass_kernel_spmd

    try:  # tracing needs the NTFF hook; without it BASS_TRACE=1 would crash
        import antenv.axon_hooks  # noqa: F401
    except ImportError:
        os.environ["BASS_NEVER_TRACE"] = "1"

    bins, contrib, mask_tiles = _derive_schedule(attn_mask)
    qt, kt, vp, use_pad, pcs = _prep_inputs(q, k, v, attn_mask, pad_mask)
    n_masks = 0 if mask_tiles is None else mask_tiles.shape[1]

    key = (np.asarray(attn_mask).tobytes(), use_pad)
    nc = _CACHE.get(key)
    if nc is None:
        nc = _build_program(bins, contrib, n_masks, use_pad)
        _CACHE[key] = nc

    in_maps = []
    for c in range(NCORES):
        sl = slice(c * HPC, (c + 1) * HPC)
        m = {"qt": qt[sl], "kt": kt[sl], "vp": vp[sl]}
        if n_masks:
            m["mk"] = mask_tiles
        if use_pad:
            m["pc"] = pcs[c]
        in_maps.append(m)

    res = run_bass_kernel_spmd(nc, in_maps, core_ids=list(range(NCORES)))
    LAST_RESULTS = res
    out = np.concatenate([res.results[c]["out"] for c in range(NCORES)], axis=0)
    return np.ascontiguousarray(out.reshape(B, H, S, D).astype(np.float32))


# revision 12
# speedup vs baseline: 1.0443x; 1.0443x over previous
"""Multi-head attention (B=4, H=16, S=2048, D=128, causal+pad mask) on 8 TRN2 NeuronCores.

Sharding: the 64 (batch, head) pairs are split 8 per core (pure data parallel —
attention is independent per head, no collectives needed).

Per-core kernel (per head):
  - scores are computed TRANSPOSED: S^T[k, q] = K_block^T^T @ Q^T with the
    contraction dim d=128 on partitions, the k-block (128) as the PSUM partition
    dim and the allowed q-columns (128-chunk granular, derived from the actual
    mask) as the moving dim. Q/K are host-cast to bf16.
  - The allowed 128x128 chunks are packed into PSUM group tiles of up to 12
    chunks ([128, 1536] f32 = 3 banks) so ONE scalar-engine ACTIVATE computes
    exp(scale*s) for the whole group out of PSUM into SBUF bf16 — the ACT
    engine is the critical path (1 col/cycle @1.2GHz), so group count is
    minimized. Bins never span more than 2 q-megatiles so only 2 PV output
    tiles are ever live. No max-subtraction: scores*scale ~ N(0,1), exp is safe.
  - Partially-masked 128x128 chunks are zeroed by a bf16 multiply with
    host-derived deduped mask tiles on the vector engine. Fully-masked chunks
    are never computed; fully-allowed chunks are untouched.
  - P^T lands exactly in the layout the PV matmul needs (k on partitions):
    O[q_sub 128, 132] += P^T[:, chunk]^T @ V'[k_block] accumulated over k
    blocks in PSUM, where V' is V in bf16 with a ones column appended at col
    128 — so O[:, 128] is the softmax denominator for free. Both q_subs of a
    256-wide megatile share one PSUM bank ([128, 2, 132]).
  - reciprocal + per-partition scale normalizes, then DMA out as f32.
  - inputs stream per head (quartered, 2-head lookahead); output DMAs trigger
    from the idle gpsimd sequencer because each dma_start costs ~620ns of
    sequencer issue time and the sync queue fits only ~27 triggers per head.
"""

import os
import sys
from collections import defaultdict

import numpy as np

try:  # the repo root that provides `concourse` / `gauge`
    import concourse.bass  # noqa: F401
except ImportError:  # pragma: no cover
    for _p in ("/opt/trn_rl_repo", "/root/.axon_site/_ro/trn_rl_repo"):
        if os.path.isdir(_p) and _p not in sys.path:
            sys.path.insert(0, _p)

import ml_dtypes

B, H, S, D = 4, 16, 2048, 128
BH = B * H
NCORES = 8
HPC = BH // NCORES  # heads per core = 8
QM = 256  # q megatile width; q sub-chunks of 128 map to PV output tiles
CH = 128  # q chunk granularity (PV stationary width / mask tile width)
KB = 128  # k block (PSUM partition dim of S^T)
NM = S // QM  # 8 q megatiles
NKB = S // KB  # 16 k blocks
VW = D + 4  # V' width: col D holds ones (softmax denom), cols D+1.. are zero pad
GCH = 12  # exp group size in chunks: [128, 1536] f32 = 3 PSUM banks
SCALE = float(np.float32(1.0 / np.sqrt(np.float32(D))))
NSUB = QM // CH  # q sub-chunks per megatile = 2

_CACHE: dict = {}
LAST_RESULTS = None  # BassKernelResults of the most recent run (for test harness)


def _derive_schedule(attn_mask):
    """Derive the chunk-level block schedule from the actual mask.

    Returns (bins, contrib, mask_tiles):
      bins: list of exp-group bins; each bin is an ordered list of chunks
        (m, j, lo, mask_id_or_None) where lo is the 128-aligned q-column
        offset within megatile m and mask_id indexes mask_tiles (None = fully
        allowed). Bins hold <= GCH chunks and never span more than 2 distinct
        megatiles (so only 2 PV accumulators are live at once).
      contrib: {(m, sub): n} count of PV contributions per output sub-tile.
      mask_tiles: [128, n_masks, CH] bf16 deduped transposed chunk masks.
    """
    am = np.asarray(attn_mask) != 0  # [S(q), S(k)]
    uniq: dict = {}
    tiles = []
    chunks = []  # (m, j, lo, mask_id|None)
    contrib: dict = defaultdict(int)
    for m in range(NM):
        for j in range(NKB):
            for c in range(NSUB):
                cm = am[m * QM + c * CH : m * QM + (c + 1) * CH, j * KB : (j + 1) * KB]
                if not cm.any():
                    continue
                if cm.all():
                    mid = None
                else:
                    key = cm.tobytes()
                    if key not in uniq:
                        uniq[key] = len(tiles)
                        tiles.append(cm.T.astype(ml_dtypes.bfloat16))  # [KB, CH]
                    mid = uniq[key]
                chunks.append((m, j, c * CH, mid))
                contrib[(m, c)] += 1

    # pack chunks into bins of exactly GCH (one ACT instruction each)
    bins = [chunks[i : i + GCH] for i in range(0, len(chunks), GCH)]

    mask_tiles = np.stack(tiles, axis=1) if tiles else None  # [128, n, CH]
    return bins, dict(contrib), mask_tiles


def _build_program(bins, contrib, n_masks, use_pad):
    import concourse.mybir as mybir
    import concourse.tile as tile
    from concourse import bacc

    f32 = mybir.dt.float32
    bf16 = mybir.dt.bfloat16
    Exp = mybir.ActivationFunctionType.Exp

    GCOLS = GCH * CH

    nc = bacc.Bacc(None)
    qt_ext = nc.declare_dram_parameter("qt", [HPC, 128, S], bf16, isOutput=False)
    kt_ext = nc.declare_dram_parameter("kt", [HPC, 128, S], bf16, isOutput=False)
    vp_ext = nc.declare_dram_parameter("vp", [HPC, 128, NKB, VW], bf16, isOutput=False)
    if n_masks:
        mk_ext = nc.declare_dram_parameter("mk", [128, n_masks, CH], bf16, isOutput=False)
    if use_pad:
        pc_ext = nc.declare_dram_parameter("pc", [128, NKB], f32, isOutput=False)
    out_ext = nc.declare_dram_parameter("out", [HPC, S, D], f32, isOutput=True)

    with tile.TileContext(nc) as tc:
        with (
            tc.tile_pool(name="qt", bufs=3) as qt_pool,
            tc.tile_pool(name="kt", bufs=3) as kt_pool,
            tc.tile_pool(name="vp", bufs=3) as vp_pool,
            tc.tile_pool(name="pt", bufs=10) as pt_pool,
            tc.tile_pool(name="osb", bufs=6) as osb_pool,
            tc.tile_pool(name="rec", bufs=4) as rec_pool,
            tc.tile_pool(name="mk", bufs=1) as mk_pool,
            tc.tile_pool(name="warm", bufs=1) as warm_pool,
            tc.tile_pool(name="st", bufs=2, space="PSUM") as st_pool,
            tc.tile_pool(name="ops", bufs=2, space="PSUM") as o_pool,
        ):
            # mask / pad-column loads first — tiny, and PV needs them early
            if n_masks:
                mk = mk_pool.tile([128, n_masks, CH], bf16)
                nc.sync.dma_start(mk[:], mk_ext[:])
            if use_pad:
                pc = mk_pool.tile([128, NKB], f32)
                nc.sync.dma_start(pc[:], pc_ext[:])

            # PE warm-up: ~3us of dummy matmuls during the DMA prologue trips
            # the HAM clock gate to 2.4 GHz before the first real matmul
            warm = warm_pool.tile([128, 512], bf16, name="warm")
            nc.gpsimd.memset(warm[:], 0.0)
            wo = st_pool.tile([128, GCOLS], f32, tag="st", name="wo")
            for wi in range(7):
                nc.tensor.matmul(
                    wo[:, 0:512], lhsT=warm[:, 0:128], rhs=warm[:], start=True, stop=True
                )

            NQ = 4  # input DMA quarters — spread across queues, compute starts early
            qs = S // NQ
            js = NKB // NQ
            for h in range(HPC):
                qt = qt_pool.tile([128, S], bf16, name="qt")
                kt = kt_pool.tile([128, S], bf16, name="kt")
                vp = vp_pool.tile([128, NKB, VW], bf16, name="vp")
                for q4 in range(NQ):
                    nc.sync.dma_start(
                        kt[:, q4 * qs : (q4 + 1) * qs],
                        kt_ext[h, :, q4 * qs : (q4 + 1) * qs],
                    )
                    nc.sync.dma_start(
                        qt[:, q4 * qs : (q4 + 1) * qs],
                        qt_ext[h, :, q4 * qs : (q4 + 1) * qs],
                    )
                    nc.sync.dma_start(
                        vp[:, q4 * js : (q4 + 1) * js, :],
                        vp_ext[h, :, q4 * js : (q4 + 1) * js, :],
                    )
                o_tiles: dict = {}
                seen: dict = defaultdict(int)
                deferred: dict = defaultdict(list)

                def finalize(m, sub, o):
                    rec = rec_pool.tile([128, 1], f32, name="rec")
                    nc.vector.reciprocal(rec[:], o[:, D : D + 1])
                    osb = osb_pool.tile([128, D], f32, name="osb")
                    nc.vector.tensor_scalar_mul(osb[:], o[:, 0:D], rec[:])
                    row0 = m * QM + sub * CH
                    # output DMAs trigger from the (otherwise idle) gpsimd
                    # sequencer: each dma_start costs ~620ns of sequencer issue
                    # time, and 28 triggers/head would saturate the sync queue
                    nc.gpsimd.dma_start(out_ext[h, row0 : row0 + CH, :], osb[:])

                def emit_pv(m, sub, j, pt, pcol):
                    # PSUM banks support only ONE open accumulation group at a
                    # time, so each (m, sub) group runs in its own 1-bank tile;
                    # sub1's group (same pool slot cycle) opens only after
                    # sub0's closes (see emit_bin's deferral).
                    key = (m, sub)
                    if key not in o_tiles:
                        o_tiles[key] = o_pool.tile([128, VW], f32, tag="o", name="o")
                    seen[key] += 1
                    nc.tensor.matmul(
                        o_tiles[key][:],
                        lhsT=pt[:, pcol : pcol + CH],
                        rhs=vp[:, j, :],
                        start=seen[key] == 1,
                        stop=seen[key] == contrib[key],
                    )
                    if seen[key] == contrib[key]:
                        finalize(m, sub, o_tiles.pop(key))
                        return True
                    return False

                def emit_bin(bn):
                    gcols = len(bn) * CH
                    st = st_pool.tile([128, GCOLS], f32, tag="st", name="st")
                    # scores: coalesce consecutive chunks of the same (m, j)
                    # into one matmul, splitting at PSUM 512-col banks
                    p = 0
                    while p < len(bn):
                        m, j, lo, _ = bn[p]
                        p2 = p + 1
                        while (
                            p2 < len(bn)
                            and bn[p2][0] == m
                            and bn[p2][1] == j
                            and bn[p2][2] == bn[p2 - 1][2] + CH
                        ):
                            p2 += 1
                        w = (p2 - p) * CH
                        off = 0
                        while off < w:
                            pcol = p * CH + off
                            wseg = min(w - off, 512 - pcol % 512)
                            nc.tensor.matmul(
                                st[:, pcol : pcol + wseg],
                                lhsT=kt[:, j * KB : (j + 1) * KB],
                                rhs=qt[:, m * QM + lo + off : m * QM + lo + off + wseg],
                                start=True,
                                stop=True,
                            )
                            off += wseg
                        p = p2
                    pt = pt_pool.tile([128, GCOLS], bf16, tag="pt", name="pt")
                    nc.scalar.activation(pt[:, :gcols], st[:, :gcols], Exp, scale=SCALE)

                    # mask/pad fixups in place, then PV. sub 0 accumulates as
                    # chunks arrive; sub 1 chunks are deferred until sub 0's
                    # accumulation group closes, so only one group per megatile
                    # (plus the neighbor's) is ever open -> 2 PSUM banks.
                    for p, (m, j, lo, mid) in enumerate(bn):
                        pcol = p * CH
                        if mid is not None:
                            nc.vector.tensor_mul(
                                pt[:, pcol : pcol + CH],
                                pt[:, pcol : pcol + CH],
                                mk[:, mid, :],
                            )
                        if use_pad:
                            nc.vector.tensor_scalar_mul(
                                pt[:, pcol : pcol + CH],
                                pt[:, pcol : pcol + CH],
                                pc[:, j : j + 1],
                            )
                        sub = lo // CH
                        if sub == 0:
                            if emit_pv(m, 0, j, pt, pcol):
                                for dm, dj, dpt, dpcol in deferred.pop(m, []):
                                    emit_pv(dm, 1, dj, dpt, dpcol)
                        elif (m, 0) in o_tiles or seen[(m, 0)] < contrib.get(
                            (m, 0), 0
                        ):
                            deferred[m].append((m, j, pt, pcol))
                        else:
                            emit_pv(m, 1, j, pt, pcol)

                for bn in bins:
                    emit_bin(bn)
    nc.compile()
    return nc


def _prep_inputs(q, k, v, attn_mask, pad_mask):
    q = np.asarray(q, dtype=np.float32).reshape(BH, S, D)
    k = np.asarray(k, dtype=np.float32).reshape(BH, S, D)
    v = np.asarray(v, dtype=np.float32).reshape(BH, S, D)

    qt = np.ascontiguousarray(q.transpose(0, 2, 1)).astype(ml_dtypes.bfloat16)
    kt = np.ascontiguousarray(k.transpose(0, 2, 1)).astype(ml_dtypes.bfloat16)

    # V': [BH, 128(row within k block), NKB, VW] bf16; col D = 1.0 (denominator)
    vp = np.zeros((BH, 128, NKB, VW), dtype=ml_dtypes.bfloat16)
    vblocks = v.reshape(BH, NKB, 128, D).transpose(0, 2, 1, 3)
    vp[:, :, :, :D] = vblocks.astype(ml_dtypes.bfloat16)
    vp[:, :, :, D] = 1.0

    pad = np.asarray(pad_mask).reshape(B, S) != 0
    use_pad = not bool(pad.all())
    pcs = None
    if use_pad:
        pcs = []
        for c in range(NCORES):
            b = (c * HPC) // H
            pcs.append(
                np.ascontiguousarray(pad[b].reshape(NKB, 128).T.astype(np.float32))
            )
    return qt, kt, vp, use_pad, pcs


def kernel(q, k, v, attn_mask, pad_mask):
    global LAST_RESULTS
    from concourse.bass_utils import run_bass_kernel_spmd

    try:  # tracing needs the NTFF hook; without it BASS_TRACE=1 would crash
        import antenv.axon_hooks  # noqa: F401
    except ImportError:
        os.environ["BASS_NEVER_TRACE"] = "1"

    bins, contrib, mask_tiles = _derive_schedule(attn_mask)
    qt, kt, vp, use_pad, pcs = _prep_inputs(q, k, v, attn_mask, pad_mask)
    n_masks = 0 if mask_tiles is None else mask_tiles.shape[1]

    key = (np.asarray(attn_mask).tobytes(), use_pad)
    nc = _CACHE.get(key)
    if nc is None:
        nc = _build_program(bins, contrib, n_masks, use_pad)
        _CACHE[key] = nc

    in_maps = []
    for c in range(NCORES):
        sl = slice(c * HPC, (c + 1) * HPC)
        m = {"qt": qt[sl], "kt": kt[sl], "vp": vp[sl]}
        if n_masks:
            m["mk"] = mask_tiles
        if use_pad:
            m["pc"] = pcs[c]
        in_maps.append(m)

    res = run_bass_kernel_spmd(nc, in_maps, core_ids=list(range(NCORES)))
    LAST_RESULTS = res
    out = np.concatenate([res.results[c]["out"] for c in range(NCORES)], axis=0)
    return np.ascontiguousarray(out.reshape(B, H, S, D).astype(np.float32))


# revision 13
# speedup vs baseline: 1.1056x; 1.0587x over previous
"""Multi-head attention (B=4, H=16, S=2048, D=128, causal+pad mask) on 8 TRN2 NeuronCores.

Sharding: the 64 (batch, head) pairs are split 8 per core (pure data parallel —
attention is independent per head, no collectives needed).

Per-core kernel (per head):
  - scores are computed TRANSPOSED: S^T[k, q] = K_block^T^T @ Q^T with the
    contraction dim d=128 on partitions, the k-block (128) as the PSUM partition
    dim and the allowed q-columns (128-chunk granular, derived from the actual
    mask) as the moving dim. Q/K are host-cast to bf16.
  - The allowed 128x128 chunks are packed into PSUM group tiles of up to 12
    chunks ([128, 1536] f32 = 3 banks) so ONE scalar-engine ACTIVATE computes
    exp(scale*s) for the whole group out of PSUM into SBUF bf16 — the ACT
    engine is the critical path (1 col/cycle @1.2GHz), so group count is
    minimized. Bins never span more than 2 q-megatiles so only 2 PV output
    tiles are ever live. No max-subtraction: scores*scale ~ N(0,1), exp is safe.
  - Partially-masked 128x128 chunks are zeroed by a bf16 multiply with
    host-derived deduped mask tiles on the vector engine. Fully-masked chunks
    are never computed; fully-allowed chunks are untouched.
  - P^T lands exactly in the layout the PV matmul needs (k on partitions):
    O[q_sub 128, 132] += P^T[:, chunk]^T @ V'[k_block] accumulated over k
    blocks in PSUM, where V' is V in bf16 with a ones column appended at col
    128 — so O[:, 128] is the softmax denominator for free. Both q_subs of a
    256-wide megatile share one PSUM bank ([128, 2, 132]).
  - reciprocal + per-partition scale normalizes, then DMA out as f32.
  - inputs stream per head (quartered, 2-head lookahead); output DMAs trigger
    from the idle gpsimd sequencer because each dma_start costs ~620ns of
    sequencer issue time and the sync queue fits only ~27 triggers per head.
"""

import os
import sys
from collections import defaultdict

import numpy as np

try:  # the repo root that provides `concourse` / `gauge`
    import concourse.bass  # noqa: F401
except ImportError:  # pragma: no cover
    for _p in ("/opt/trn_rl_repo", "/root/.axon_site/_ro/trn_rl_repo"):
        if os.path.isdir(_p) and _p not in sys.path:
            sys.path.insert(0, _p)

import ml_dtypes

B, H, S, D = 4, 16, 2048, 128
BH = B * H
NCORES = 8
HPC = BH // NCORES  # heads per core = 8
QM = 256  # q megatile width; q sub-chunks of 128 map to PV output tiles
CH = 128  # q chunk granularity (PV stationary width / mask tile width)
KB = 128  # k block (PSUM partition dim of S^T)
NM = S // QM  # 8 q megatiles
NKB = S // KB  # 16 k blocks
VW = D + 4  # V' width: col D holds ones (softmax denom), cols D+1.. are zero pad
GCH = 12  # exp group size in chunks: [128, 1536] f32 = 3 PSUM banks
SCALE = float(np.float32(1.0 / np.sqrt(np.float32(D))))
NSUB = QM // CH  # q sub-chunks per megatile = 2

_CACHE: dict = {}
LAST_RESULTS = None  # BassKernelResults of the most recent run (for test harness)


def _derive_schedule(attn_mask):
    """Derive the chunk-level block schedule from the actual mask.

    Returns (bins, contrib, mask_tiles):
      bins: list of exp-group bins; each bin is an ordered list of chunks
        (m, j, lo, mask_id_or_None) where lo is the 128-aligned q-column
        offset within megatile m and mask_id indexes mask_tiles (None = fully
        allowed). Bins hold <= GCH chunks and never span more than 2 distinct
        megatiles (so only 2 PV accumulators are live at once).
      contrib: {(m, sub): n} count of PV contributions per output sub-tile.
      mask_tiles: [128, n_masks, CH] bf16 deduped transposed chunk masks.
    """
    am = np.asarray(attn_mask) != 0  # [S(q), S(k)]
    uniq: dict = {}
    tiles = []
    chunks = []  # (m, j, lo, mask_id|None)
    contrib: dict = defaultdict(int)
    for m in range(NM):
        for j in range(NKB):
            for c in range(NSUB):
                cm = am[m * QM + c * CH : m * QM + (c + 1) * CH, j * KB : (j + 1) * KB]
                if not cm.any():
                    continue
                if cm.all():
                    mid = None
                else:
                    key = cm.tobytes()
                    if key not in uniq:
                        uniq[key] = len(tiles)
                        tiles.append(cm.T.astype(ml_dtypes.bfloat16))  # [KB, CH]
                    mid = uniq[key]
                chunks.append((m, j, c * CH, mid))
                contrib[(m, c)] += 1

    # pack chunks into bins of exactly GCH (one ACT instruction each)
    bins = [chunks[i : i + GCH] for i in range(0, len(chunks), GCH)]

    mask_tiles = np.stack(tiles, axis=1) if tiles else None  # [128, n, CH]
    return bins, dict(contrib), mask_tiles


def _build_program(bins, contrib, n_masks, use_pad):
    import concourse.mybir as mybir
    import concourse.tile as tile
    from concourse import bacc

    f32 = mybir.dt.float32
    bf16 = mybir.dt.bfloat16
    Exp = mybir.ActivationFunctionType.Exp

    GCOLS = GCH * CH

    nc = bacc.Bacc(None)
    qt_ext = nc.declare_dram_parameter("qt", [HPC, 128, S], bf16, isOutput=False)
    kt_ext = nc.declare_dram_parameter("kt", [HPC, 128, S], bf16, isOutput=False)
    vp_ext = nc.declare_dram_parameter("vp", [HPC, 128, NKB, VW], bf16, isOutput=False)
    if n_masks:
        mk_ext = nc.declare_dram_parameter("mk", [128, n_masks, CH], bf16, isOutput=False)
    if use_pad:
        pc_ext = nc.declare_dram_parameter("pc", [128, NKB], f32, isOutput=False)
    out_ext = nc.declare_dram_parameter("out", [HPC, S, D], f32, isOutput=True)

    with tile.TileContext(nc) as tc:
        with (
            tc.tile_pool(name="qt", bufs=3) as qt_pool,
            tc.tile_pool(name="kt", bufs=3) as kt_pool,
            tc.tile_pool(name="vp", bufs=3) as vp_pool,
            tc.tile_pool(name="pt", bufs=10) as pt_pool,
            tc.tile_pool(name="osb", bufs=6) as osb_pool,
            tc.tile_pool(name="rec", bufs=4) as rec_pool,
            tc.tile_pool(name="mk", bufs=1) as mk_pool,
            tc.tile_pool(name="warm", bufs=1) as warm_pool,
            tc.tile_pool(name="st", bufs=2, space="PSUM") as st_pool,
            tc.tile_pool(name="ops", bufs=2, space="PSUM") as o_pool,
        ):
            # mask / pad-column loads first — tiny, and PV needs them early
            if n_masks:
                mk = mk_pool.tile([128, n_masks, CH], bf16)
                nc.sync.dma_start(mk[:], mk_ext[:])
            if use_pad:
                pc = mk_pool.tile([128, NKB], f32)
                nc.sync.dma_start(pc[:], pc_ext[:])

            # PE warm-up: ~3us of dummy matmuls during the DMA prologue trips
            # the HAM clock gate to 2.4 GHz before the first real matmul
            warm = warm_pool.tile([128, 512], bf16, name="warm")
            nc.gpsimd.memset(warm[:], 0.0)
            wo = st_pool.tile([128, GCOLS], f32, tag="st", name="wo")
            for wi in range(7):
                nc.tensor.matmul(
                    wo[:, 0:512], lhsT=warm[:, 0:128], rhs=warm[:], start=True, stop=True
                )

            NQ = 4  # input DMA quarters — spread across queues, compute starts early
            qs = S // NQ
            js = NKB // NQ

            class Head:
                def __init__(self, h):
                    self.h = h
                    self.qt = qt_pool.tile([128, S], bf16, name="qt")
                    self.kt = kt_pool.tile([128, S], bf16, name="kt")
                    self.vp = vp_pool.tile([128, NKB, VW], bf16, name="vp")
                    for q4 in range(NQ):
                        nc.sync.dma_start(
                            self.kt[:, q4 * qs : (q4 + 1) * qs],
                            kt_ext[h, :, q4 * qs : (q4 + 1) * qs],
                        )
                        nc.sync.dma_start(
                            self.qt[:, q4 * qs : (q4 + 1) * qs],
                            qt_ext[h, :, q4 * qs : (q4 + 1) * qs],
                        )
                        nc.sync.dma_start(
                            self.vp[:, q4 * js : (q4 + 1) * js, :],
                            vp_ext[h, :, q4 * js : (q4 + 1) * js, :],
                        )
                    self.o_tiles: dict = {}
                    self.seen: dict = defaultdict(int)
                    self.deferred: dict = defaultdict(list)

            def finalize(hd, m, sub, o):
                rec = rec_pool.tile([128, 1], f32, name="rec")
                nc.vector.reciprocal(rec[:], o[:, D : D + 1])
                osb = osb_pool.tile([128, D], f32, name="osb")
                nc.vector.tensor_scalar_mul(osb[:], o[:, 0:D], rec[:])
                row0 = m * QM + sub * CH
                # output DMAs trigger from the (otherwise idle) gpsimd
                # sequencer: each dma_start costs ~620ns of sequencer issue
                # time, and 28 triggers/head would saturate the sync queue
                nc.gpsimd.dma_start(out_ext[hd.h, row0 : row0 + CH, :], osb[:])

            def emit_pv(hd, m, sub, j, pt, pcol):
                # PSUM banks support only ONE open accumulation group at a
                # time, so each (m, sub) group runs in its own 1-bank tile;
                # sub1's group (same pool slot cycle) opens only after
                # sub0's closes (see the deferral in emit_pv_phase).
                key = (m, sub)
                if key not in hd.o_tiles:
                    hd.o_tiles[key] = o_pool.tile([128, VW], f32, tag="o", name="o")
                hd.seen[key] += 1
                nc.tensor.matmul(
                    hd.o_tiles[key][:],
                    lhsT=pt[:, pcol : pcol + CH],
                    rhs=hd.vp[:, j, :],
                    start=hd.seen[key] == 1,
                    stop=hd.seen[key] == contrib[key],
                )
                if hd.seen[key] == contrib[key]:
                    finalize(hd, m, sub, hd.o_tiles.pop(key))
                    return True
                return False

            def emit_scores_phase(hd, bn):
                gcols = len(bn) * CH
                st = st_pool.tile([128, GCOLS], f32, tag="st", name="st")
                # scores: coalesce consecutive chunks of the same (m, j)
                # into one matmul, splitting at PSUM 512-col banks
                p = 0
                while p < len(bn):
                    m, j, lo, _ = bn[p]
                    p2 = p + 1
                    while (
                        p2 < len(bn)
                        and bn[p2][0] == m
                        and bn[p2][1] == j
                        and bn[p2][2] == bn[p2 - 1][2] + CH
                    ):
                        p2 += 1
                    w = (p2 - p) * CH
                    off = 0
                    while off < w:
                        pcol = p * CH + off
                        wseg = min(w - off, 512 - pcol % 512)
                        nc.tensor.matmul(
                            st[:, pcol : pcol + wseg],
                            lhsT=hd.kt[:, j * KB : (j + 1) * KB],
                            rhs=hd.qt[
                                :, m * QM + lo + off : m * QM + lo + off + wseg
                            ],
                            start=True,
                            stop=True,
                        )
                        off += wseg
                    p = p2
                pt = pt_pool.tile([128, GCOLS], bf16, tag="pt", name="pt")
                nc.scalar.activation(pt[:, :gcols], st[:, :gcols], Exp, scale=SCALE)
                return pt

            def emit_pv_phase(hd, bn, pt):
                # mask/pad fixups in place, then PV. sub 0 accumulates as
                # chunks arrive; sub 1 chunks are deferred until sub 0's
                # accumulation group closes, so only one group per megatile
                # (plus the neighbor's) is ever open -> 2 PSUM banks.
                for p, (m, j, lo, mid) in enumerate(bn):
                    pcol = p * CH
                    if mid is not None:
                        nc.vector.tensor_mul(
                            pt[:, pcol : pcol + CH],
                            pt[:, pcol : pcol + CH],
                            mk[:, mid, :],
                        )
                    if use_pad:
                        nc.vector.tensor_scalar_mul(
                            pt[:, pcol : pcol + CH],
                            pt[:, pcol : pcol + CH],
                            pc[:, j : j + 1],
                        )
                    sub = lo // CH
                    if sub == 0:
                        if emit_pv(hd, m, 0, j, pt, pcol):
                            for dj, dpt, dpcol in hd.deferred.pop(m, []):
                                emit_pv(hd, m, 1, dj, dpt, dpcol)
                    elif (m, 0) in hd.o_tiles or hd.seen[(m, 0)] < contrib.get(
                        (m, 0), 0
                    ):
                        hd.deferred[m].append((j, pt, pcol))
                    else:
                        emit_pv(hd, m, 1, j, pt, pcol)

            # software-pipeline by one bin ACROSS heads: emit scores+exp of
            # bin g, then the PV batch of bin g-1 — so every PV-only stretch
            # on the PE (especially the deferred sub1 flushes) overlaps an
            # in-flight exp and the ACT engine never starves.
            heads = {}
            flat = [(h, bn) for h in range(HPC) for bn in bins]
            prev = None
            for h, bn in flat:
                if h not in heads:
                    heads[h] = Head(h)
                pt = emit_scores_phase(heads[h], bn)
                if prev is not None:
                    emit_pv_phase(*prev)
                prev = (heads[h], bn, pt)
            emit_pv_phase(*prev)
    nc.compile()
    return nc


def _prep_inputs(q, k, v, attn_mask, pad_mask):
    q = np.asarray(q, dtype=np.float32).reshape(BH, S, D)
    k = np.asarray(k, dtype=np.float32).reshape(BH, S, D)
    v = np.asarray(v, dtype=np.float32).reshape(BH, S, D)

    qt = np.ascontiguousarray(q.transpose(0, 2, 1)).astype(ml_dtypes.bfloat16)
    kt = np.ascontiguousarray(k.transpose(0, 2, 1)).astype(ml_dtypes.bfloat16)

    # V': [BH, 128(row within k block), NKB, VW] bf16; col D = 1.0 (denominator)
    vp = np.zeros((BH, 128, NKB, VW), dtype=ml_dtypes.bfloat16)
    vblocks = v.reshape(BH, NKB, 128, D).transpose(0, 2, 1, 3)
    vp[:, :, :, :D] = vblocks.astype(ml_dtypes.bfloat16)
    vp[:, :, :, D] = 1.0

    pad = np.asarray(pad_mask).reshape(B, S) != 0
    use_pad = not bool(pad.all())
    pcs = None
    if use_pad:
        pcs = []
        for c in range(NCORES):
            b = (c * HPC) // H
            pcs.append(
                np.ascontiguousarray(pad[b].reshape(NKB, 128).T.astype(np.float32))
            )
    return qt, kt, vp, use_pad, pcs


def kernel(q, k, v, attn_mask, pad_mask):
    global LAST_RESULTS
    from concourse.bass_utils import run_bass_kernel_spmd

    try:  # tracing needs the NTFF hook; without it BASS_TRACE=1 would crash
        import antenv.axon_hooks  # noqa: F401
    except ImportError:
        os.environ["BASS_NEVER_TRACE"] = "1"

    bins, contrib, mask_tiles = _derive_schedule(attn_mask)
    qt, kt, vp, use_pad, pcs = _prep_inputs(q, k, v, attn_mask, pad_mask)
    n_masks = 0 if mask_tiles is None else mask_tiles.shape[1]

    key = (np.asarray(attn_mask).tobytes(), use_pad)
    nc = _CACHE.get(key)
    if nc is None:
        nc = _build_program(bins, contrib, n_masks, use_pad)
        _CACHE[key] = nc

    in_maps = []
    for c in range(NCORES):
        sl = slice(c * HPC, (c + 1) * HPC)
        m = {"qt": qt[sl], "kt": kt[sl], "vp": vp[sl]}
        if n_masks:
            m["mk"] = mask_tiles
        if use_pad:
            m["pc"] = pcs[c]
        in_maps.append(m)

    res = run_bass_kernel_spmd(nc, in_maps, core_ids=list(range(NCORES)))
    LAST_RESULTS = res
    out = np.concatenate([res.results[c]["out"] for c in range(NCORES)], axis=0)
    return np.ascontiguousarray(out.reshape(B, H, S, D).astype(np.float32))
